# revision 11
# baseline (speedup 1.0000x reference)
"""AttnBlock (GroupNorm -> single-head self-attention -> residual) on 8 TRN2 cores.

Sharding: B=4 batch elements x 2 query-token halves = 8 cores (SPMD, no
collectives).  Each core receives the full (rolled) channel-major batch
element x^T [C=256, HW=4096] in bf16, computes GroupNorm + k/v for all
4096 tokens, and q/scores/attention/out-proj for its 2048-token half.
Odd cores get x rolled by -2048 tokens; attention is permutation-
invariant over keys, so their first 2048 tokens are tokens 2048:4096.

The two big attention matmuls (scores and attn@v) and the softmax-
denominator chain run in fp8-e4m3 with MatmulPerfMode.DoubleRow
(K=256 packed 2-rows-per-PE-cell, 0.5 cycles/row).  Softmax numerators
use exp(s/16 - 2) so es <= ~57 < 240 (TRN fp8e4 max); the constant
offset cancels in the softmax ratio.  Projections are bf16.  Layout is
channel-major (tokens on the free axis), all matmuls transpose-free:

  hs^T = GN(x^T)  bf16                    [C, N]
  q^T = Wq^T.T @ hs^T -> fp8 (ACT cast)   [C, NQ]   (dim1 = ko ktile)
  k^T likewise -> fp8                     [C, N]
  v   = hs^T.T @ Wv^T + bv -> fp8         [N, C]    (row-major)
  S^T = DR(k^T, q^T)                      [N, NQ]   one matmul per m-tile
  es  = exp(S^T/16 - 2) -> fp8 (ACT)
  Z   = DR(ones, es) chain                [16, NQ]  (row 0 used)
  o^T = DR(v, es) chain                   [C, NQ]
  out^T = (Wo^T*2^-0.5).T @ bf16(o^T)     [C, NQ]
  final = xr + out^T * (1/Z),  xr = (x + bo) * 2^-0.5  (host-side)
"""

import numpy as np
import ml_dtypes

import concourse.bass as bass
import concourse.tile as tile
from concourse import bacc, mybir
from concourse.bass_utils import run_bass_kernel_spmd

dt = mybir.dt
F32, F32R, BF16, FP8 = dt.float32, dt.float32r, dt.bfloat16, dt.float8e4
AF = mybir.ActivationFunctionType
ALU = mybir.AluOpType
DR = mybir.MatmulPerfMode.DoubleRow

P = 128          # partitions
C = 256          # channels
N = 4096         # tokens per batch element (64*64)
NQ = 2048        # query tokens per core
NSTRIP = 256     # query-token strip width
NS = NQ // NSTRIP  # 8 strips
MT = N // P      # 32 key m-tiles
GS = 8           # channels per group (256 / 32 groups)
EPS = 1e-6
ISCALE = 1.0 / 16.0       # attention scale c**-0.5
EOFF = 2.0                # exp offset: es = exp(s/16 - EOFF), cancels in softmax
RS2 = float(2.0 ** -0.5)  # output residual scale

import os
DR_SCORES = os.environ.get("K_DR_SCORES", "1") == "1"
DR_ATTNV = os.environ.get("K_DR_ATTNV", "1") == "1"
DR_Z = os.environ.get("K_DR_Z", "1") == "1"
ACT_QK = os.environ.get("K_ACT_QK", "1") == "1"

_prog_cache = {}


def _build_nc():
    nc = bacc.Bacc("TRN2", target_bir_lowering=False, debug=False, num_devices=8)

    def inp(name, shape, d=F32):
        return nc.dram_tensor(name, shape, d, kind="ExternalInput").ap()

    xtb_d = inp("xtb", [2, P, N], BF16)    # [c_half, c_in, n] bf16
    xr_d = inp("xr", [2, P, N])            # (x + bo) * 2^-0.5
    wq_d = inp("wqT", [2, P, C], BF16)     # [ci_half, ci_in, c_out] = Wq.T
    wk_d = inp("wkT", [2, P, C], BF16)
    wv_d = inp("wvT", [2, P, C], BF16)
    wo_d = inp("woT", [2, P, C], BF16)     # Wo.T * 2^-0.5
    bq_d = inp("bqp", [P, 2])              # [c_out_in, c_out_half]
    bk_d = inp("bkp", [P, 2])
    bv_d = inp("bv4", [1, 4 * C])          # bv tiled 4x for [P,4,C] broadcast
    gnw_d = inp("gnw", [P, 2])
    gnb_d = inp("gnb", [P, 2])
    amat_d = inp("amat", [P, P])           # block-diag 8x8 of 1/8
    ones1_d = inp("ones1", [1, P])
    out_d = nc.dram_tensor("out", [2, P, NQ], F32, kind="ExternalOutput").ap()

    with tile.TileContext(nc) as tc:
        with (
            tc.tile_pool(name="singles", bufs=1) as singles,
            tc.tile_pool(name="xpool", bufs=1) as xpool,
            tc.tile_pool(name="hsp", bufs=1) as hsp,
            tc.tile_pool(name="qk", bufs=1) as qk,
            tc.tile_pool(name="vpool", bufs=1) as vpool,
            tc.tile_pool(name="espool", bufs=2) as espool,
            tc.tile_pool(name="small", bufs=2) as small,
            tc.tile_pool(name="zf", bufs=2) as zf,
            tc.tile_pool(name="ps", bufs=2, space="PSUM") as ps,    # 2x2 banks
            tc.tile_pool(name="po", bufs=2, space="PSUM") as po,    # 2x1 bank
            tc.tile_pool(name="pz", bufs=1, space="PSUM") as pz,    # 1 bank
            tc.tile_pool(name="pr", bufs=1, space="PSUM") as pr,    # 1 bank
        ):
            # ---- x load first (chunked; stats pipeline behind chunks) ----
            xtb = xpool.tile([P, 2, N], BF16, tag="xtb")
            _dmae = [nc.sync, nc.scalar]
            for t in range(2):
                for h in range(2):
                    _dmae[h].dma_start(
                        xtb[:, t, h * 2048:(h + 1) * 2048],
                        xtb_d[t, :, h * 2048:(h + 1) * 2048])
            # residual input; needed late (strip tails) so queued after xtb
            xr = xpool.tile([P, 2, N], F32, tag="xr")
            for t in range(2):
                for h in range(2):
                    _dmae[h].dma_start(
                        xr[:, t, h * 2048:(h + 1) * 2048],
                        xr_d[t, :, h * 2048:(h + 1) * 2048])

            # ---- weights / constants (gpsimd software queues) ----
            wk = singles.tile([P, 2, C], BF16)
            for ko in range(2):
                nc.gpsimd.dma_start(wk[:, ko, :], wk_d[ko])
            wq = singles.tile([P, 2, C], BF16)
            for ko in range(2):
                nc.gpsimd.dma_start(wq[:, ko, :], wq_d[ko])
            wv = singles.tile([P, 2, C], BF16)
            for ko in range(2):
                nc.gpsimd.dma_start(wv[:, ko, :], wv_d[ko])
            wo = singles.tile([P, 2, C], BF16)
            for ko in range(2):
                nc.gpsimd.dma_start(wo[:, ko, :], wo_d[ko])
            bq = singles.tile([P, 2], F32)
            nc.gpsimd.dma_start(bq[:], bq_d)
            bk = singles.tile([P, 2], F32)
            nc.gpsimd.dma_start(bk[:], bk_d)
            gnw = singles.tile([P, 2], F32)
            nc.gpsimd.dma_start(gnw[:], gnw_d)
            gnb = singles.tile([P, 2], F32)
            nc.gpsimd.dma_start(gnb[:], gnb_d)
            amat = singles.tile([P, P], F32R)
            nc.gpsimd.dma_start(amat[:], amat_d.bitcast(F32R))
            ones1 = singles.tile([1, P], F32R)
            nc.gpsimd.dma_start(ones1[:], ones1_d.bitcast(F32R))
            # bv broadcast to all partitions (stride-0 partition DMA)
            bvrep = singles.tile([P, 4, C], F32)
            bv_b = bass.AP(tensor=bv_d.tensor, offset=bv_d.offset,
                           ap=[[0, P], [1, 4 * C]])
            nc.gpsimd.dma_start(out=bvrep[:].rearrange("p a b -> p (a b)"),
                                in_=bv_b)
            ones8z = singles.tile([P, 2, 16], FP8)
            nc.vector.memset(ones8z[:], 1.0)
            noff = singles.tile([P, 1], F32)
            nc.vector.memset(noff[:], -EOFF)
            epsap = singles.tile([P, 1], F32)
            nc.vector.memset(epsap[:], EPS)

            # ---- GroupNorm stats (per channel, then 8-chan group aggregate) ----
            mv2 = small.tile([P, 4], F32, tag="gnmv")  # [mu_t0 mu_t1 ex2_t0 ex2_t1]
            for t in range(2):
                st = small.tile([P, 8, 6], F32, tag="gnst", name=f"gnst{t}")
                xre = xtb[:, t, :].rearrange("p (s f) -> p s f", f=512)
                for sg in range(8):
                    nc.vector.bn_stats(st[:, sg, :], xre[:, sg, :])
                mvt = small.tile([P, 2], F32, tag="gnmvt", name=f"gnmvt{t}")
                nc.vector.bn_aggr(mvt[:], st[:])  # [mean, var]
                musq = small.tile([P, 1], F32, tag="gnmusq", name=f"gnmusq{t}")
                nc.vector.tensor_mul(musq[:], mvt[:, 0:1], mvt[:, 0:1])
                nc.vector.tensor_copy(mv2[:, t:t + 1], mvt[:, 0:1])
                nc.vector.tensor_add(mv2[:, 2 + t:3 + t], mvt[:, 1:2], musq[:])
            stats2 = small.tile([P, 4], F32R, tag="gnst2")
            nc.vector.tensor_copy(stats2[:], mv2[:])
            gp = pz.tile([P, 512], F32, tag="pz", name="gnagg")
            nc.tensor.matmul(gp[:, 0:4], amat[:], stats2[:], start=True, stop=True)
            gs = small.tile([P, 4], F32, tag="gnagg2")
            nc.vector.tensor_copy(gs[:], gp[:, 0:4])
            gmusq = small.tile([P, 2], F32, tag="gnmusq2")
            nc.vector.tensor_mul(gmusq[:], gs[:, 0:2], gs[:, 0:2])
            gvar = small.tile([P, 2], F32, tag="gnvar")
            nc.vector.tensor_tensor(gvar[:], gs[:, 2:4], gmusq[:], ALU.subtract)
            # rstd = exp(-0.5 * ln(var + eps)) (same ACT table set as softmax)
            lnv = small.tile([P, 2], F32, tag="gnln")
            nc.scalar.activation(lnv[:], gvar[:], AF.Ln, bias=epsap[:], scale=1.0)
            rstd = small.tile([P, 2], F32, tag="gnrstd")
            nc.scalar.activation(rstd[:], lnv[:], AF.Exp, bias=0.0, scale=-0.5)
            alpha = small.tile([P, 2], F32, tag="gnalpha")
            nc.vector.tensor_mul(alpha[:], rstd[:], gnw[:])
            atmp = small.tile([P, 2], F32, tag="gnatmp")
            nc.vector.tensor_mul(atmp[:], gs[:, 0:2], alpha[:])
            beta = small.tile([P, 2], F32, tag="gnbeta")
            nc.vector.tensor_tensor(beta[:], gnb[:], atmp[:], ALU.subtract)
            hs = hsp.tile([P, 2, N], BF16, tag="hs")
            for t in range(2):
                nc.vector.tensor_scalar(hs[:, t, :], xtb[:, t, :],
                                        alpha[:, t:t + 1], beta[:, t:t + 1],
                                        ALU.mult, ALU.add)

            # ---- projections: k first (strip 0 needs all of k) ----
            kT = qk.tile([P, 2, N], FP8, tag="kT")
            qT = qk.tile([P, 2, NQ], FP8, tag="qT")
            for (wt, bt, dst, ntok) in ((wk, bk, kT, N), (wq, bq, qT, NQ)):
                for ch in range(2):
                    for blk in range(ntok // 512):
                        kp = po.tile([P, 2, NSTRIP], F32, tag="po",
                                     name=f"pj{id(wt)}_{ch}_{blk}")
                        kpf = kp[:].rearrange("p a b -> p (a b)")
                        for ko in range(2):
                            nc.tensor.matmul(
                                kpf, wt[:, ko, ch * P:(ch + 1) * P],
                                hs[:, ko, blk * 512:(blk + 1) * 512],
                                start=(ko == 0), stop=(ko == 1))
                        if ACT_QK:
                            nc.scalar.activation(
                                dst[:, ch, blk * 512:(blk + 1) * 512], kpf,
                                AF.Identity, bias=bt[:, ch:ch + 1], scale=1.0)
                        else:
                            nc.vector.tensor_scalar(
                                dst[:, ch, blk * 512:(blk + 1) * 512], kpf,
                                bt[:, ch:ch + 1], None, ALU.add)
            v = vpool.tile([P, MT, C], FP8)
            for g in range(MT // 4):
                vp = ps.tile([P, 4, NSTRIP], F32, tag="ps", name=f"vp{g}")
                for i in range(4):
                    m = 4 * g + i
                    for ko in range(2):
                        nc.tensor.matmul(vp[:, i, :],
                                         hs[:, ko, m * P:(m + 1) * P],
                                         wv[:, ko, :],
                                         start=(ko == 0), stop=(ko == 1))
                nc.vector.tensor_tensor(v[:, 4 * g:4 * g + 4, :], vp[:],
                                        bvrep[:], ALU.add)

            # ---- attention strips (software-pipelined emission) ----
            es_t = [None] * NS
            zp_t = [None] * NS
            op_t = [None] * NS
            rz_t = [None] * NS
            osb_t = [None] * NS

            def emit_scores_exp(s):
                ns = slice(s * NSTRIP, (s + 1) * NSTRIP)
                es = espool.tile([P, MT, NSTRIP], FP8, tag="es", name=f"es{s}")
                es_t[s] = es
                for j in range(MT // 4):
                    sp = ps.tile([P, 4, NSTRIP], F32, tag="ps", name=f"sp{s}_{j}")
                    for i in range(4):
                        m = 4 * j + i
                        if DR_SCORES:
                            nc.tensor.matmul(sp[:, i, :],
                                             kT[:, :, m * P:(m + 1) * P],
                                             qT[:, :, ns],
                                             start=True, stop=True, perf_mode=DR)
                        else:
                            for ko in range(2):
                                nc.tensor.matmul(sp[:, i, :],
                                                 kT[:, ko, m * P:(m + 1) * P],
                                                 qT[:, ko, ns],
                                                 start=(ko == 0), stop=(ko == 1))
                    nc.scalar.activation(es[:, 4 * j:4 * j + 4, :], sp[:],
                                         AF.Exp, bias=noff[:], scale=ISCALE)

            def emit_zav(s):
                es = es_t[s]
                zp = pz.tile([P, 512], F32, tag="pz", name=f"zp{s}")
                op = po.tile([P, 2, NSTRIP], F32, tag="po", name=f"op{s}")
                zp_t[s], op_t[s] = zp, op
                # NOTE: accumulation chains must NOT interleave — the PE has a
                # single open accumulation context; interleaving groups (even
                # into different banks) corrupts the sums.
                for j2 in range(MT // 2):
                    nc.tensor.matmul(zp[0:16, 0:NSTRIP], ones8z[:],
                                     es[:, 2 * j2:2 * j2 + 2, :],
                                     start=(j2 == 0),
                                     stop=(j2 == MT // 2 - 1),
                                     perf_mode=DR)
                for ch in range(2):
                    for j2 in range(MT // 2):
                        nc.tensor.matmul(op[:, ch, :],
                                         v[:, 2 * j2:2 * j2 + 2,
                                           ch * P:(ch + 1) * P],
                                         es[:, 2 * j2:2 * j2 + 2, :],
                                         start=(j2 == 0),
                                         stop=(j2 == MT // 2 - 1),
                                         perf_mode=DR)

            def emit_tail_a(s):
                # psum reads that free zp/op for the next strip
                rz = small.tile([1, NSTRIP], F32R, tag="rz", name=f"rz{s}")
                rz_t[s] = rz
                rzf = small.tile([1, NSTRIP], F32, tag="rzf", name=f"rzf{s}")
                with nc.allow_low_precision(reason="~18-bit 1/Z is plenty"):
                    nc.vector.reciprocal_approx_fast(rzf[:], zp_t[s][0:1, 0:NSTRIP])
                    nc.vector.tensor_copy(rz[:], rzf[:])
                osb = small.tile([P, 2, NSTRIP], BF16, tag="osb", name=f"osb{s}")
                osb_t[s] = osb
                nc.vector.tensor_copy(osb[:], op_t[s][:])

            def emit_tail_b(s):
                ns = slice(s * NSTRIP, (s + 1) * NSTRIP)
                rp = pr.tile([P, 512], F32, tag="pr", name=f"rp{s}")
                nc.tensor.matmul(rp[:, 0:NSTRIP], ones1[:],
                                 rz_t[s][:], start=True, stop=True)
                op2 = po.tile([P, 2, NSTRIP], F32, tag="po", name=f"op2_{s}")
                for ch in range(2):
                    for ko in range(2):
                        nc.tensor.matmul(op2[:, ch, :],
                                         wo[:, ko, ch * P:(ch + 1) * P],
                                         osb_t[s][:, ko, :],
                                         start=(ko == 0), stop=(ko == 1))
                rzs = small.tile([P, NSTRIP], F32, tag="rzs", name=f"rzs{s}")
                nc.vector.tensor_copy(rzs[:], rp[:, 0:NSTRIP])
                tt = zf.tile([P, 2, NSTRIP], F32, tag="tt", name=f"tt{s}")
                for ch in range(2):
                    nc.vector.tensor_tensor(tt[:, ch, :], op2[:, ch, :],
                                            rzs[:], ALU.mult)
                fin = zf.tile([P, 2, NSTRIP], F32, tag="fin", name=f"fin{s}")
                nc.vector.tensor_tensor(fin[:], xr[:, :, ns], tt[:], ALU.add)
                for t in range(2):
                    nc.sync.dma_start(out_d[t, :, ns], fin[:, t, :])

            emit_scores_exp(0)
            emit_scores_exp(1)
            for s in range(NS):
                emit_zav(s)
                emit_tail_a(s)
                if s + 2 < NS:
                    emit_scores_exp(s + 2)
                emit_tail_b(s)

    nc.finalize()
    return nc


def _get_nc():
    if "nc" not in _prog_cache:
        _prog_cache["nc"] = _build_nc()
    return _prog_cache["nc"]


def _make_in_maps(x, gn_weight, gn_bias, Wq, bq, Wk, bk, Wv, bv, Wo, bo):
    x = np.asarray(x, dtype=np.float32)
    f32 = lambda a: np.ascontiguousarray(np.asarray(a, dtype=np.float32))
    b16 = lambda a: np.ascontiguousarray(
        np.asarray(a, dtype=np.float32).astype(ml_dtypes.bfloat16))

    def packT(b_vec):  # [256] -> [128, 2] (c_out_in, c_out_half)
        return np.ascontiguousarray(f32(b_vec).reshape(2, P).T)

    amat = np.zeros((P, P), np.float32)
    for g in range(P // GS):
        amat[g * GS:(g + 1) * GS, g * GS:(g + 1) * GS] = 1.0 / GS

    common = {
        "wqT": b16(np.asarray(Wq).T).reshape(2, P, C),
        "wkT": b16(np.asarray(Wk).T).reshape(2, P, C),
        "wvT": b16(np.asarray(Wv).T).reshape(2, P, C),
        "woT": b16(np.asarray(Wo, dtype=np.float32).T * RS2).reshape(2, P, C),
        "bqp": packT(bq),
        "bkp": packT(bk),
        "bv4": np.ascontiguousarray(np.tile(f32(bv).reshape(1, C), (1, 4))),
        "gnw": packT(gn_weight),
        "gnb": packT(gn_bias),
        "amat": amat,
        "ones1": np.ones((1, P), np.float32),
    }

    bo_col = f32(bo).reshape(C, 1)
    in_maps = []
    for core in range(8):
        b, half = core // 2, core % 2
        xt = x[b].reshape(C, N)
        if half:
            xt = np.roll(xt, -NQ, axis=1)
        xrm = ((xt + bo_col) * RS2).astype(np.float32)
        in_maps.append({
            "xtb": np.ascontiguousarray(
                xt.astype(ml_dtypes.bfloat16)).reshape(2, P, N),
            "xr": np.ascontiguousarray(xrm).reshape(2, P, N),
            **common,
        })
    return in_maps


def _assemble(results, B):
    out = np.empty((B, C, N), np.float32)
    for core in range(2 * B):
        b, half = core // 2, core % 2
        out[b, :, half * NQ:(half + 1) * NQ] = results[core]["out"].reshape(C, NQ)
    return out.reshape(B, C, 64, 64)


def kernel(x, gn_weight, gn_bias, Wq, bq, Wk, bk, Wv, bv, Wo, bo):
    x = np.asarray(x, dtype=np.float32)
    in_maps = _make_in_maps(x, gn_weight, gn_bias, Wq, bq, Wk, bk, Wv, bv, Wo, bo)
    nc = _get_nc()
    res = run_bass_kernel_spmd(nc, in_maps, list(range(8)))
    return _assemble(res.results, x.shape[0])


# revision 14
# speedup vs baseline: 1.0375x; 1.0375x over previous
"""AttnBlock (GroupNorm -> single-head self-attention -> residual) on 8 TRN2 cores.

Sharding: B=4 batch elements x 2 query-token halves = 8 cores (SPMD, no
collectives).  Each core receives the full (rolled) channel-major batch
element x^T [C=256, HW=4096] in bf16, computes GroupNorm + k/v for all
4096 tokens, and q/scores/attention/out-proj for its 2048-token half.
Odd cores get x rolled by -2048 tokens; attention is permutation-
invariant over keys, so their first 2048 tokens are tokens 2048:4096.

The two big attention matmuls (scores and attn@v) and the softmax-
denominator chain run in fp8-e4m3 with MatmulPerfMode.DoubleRow
(K=256 packed 2-rows-per-PE-cell, 0.5 cycles/row).  Softmax numerators
use exp(s/16 - 2) so es <= ~57 < 240 (TRN fp8e4 max); the constant
offset cancels in the softmax ratio.  Projections are bf16.  PSUM
accumulation chains never interleave (the PE has one open accumulation
context; interleaving corrupts sums).  GroupNorm rstd uses a Quake-
style rsqrt on DVE (bit-trick + 2 Newton steps) so the ACT engine only
ever loads one table set (Exp/Identity).  Layout is channel-major
(tokens on the free axis), all matmuls transpose-free:

  hs^T = GN(x^T)  bf16                    [C, N]
  q^T = Wq^T.T @ hs^T -> fp8              [C, NQ]   (dim1 = ko ktile)
  k^T likewise -> fp8                     [C, N]
  v   = hs^T.T @ Wv^T + bv -> fp8         [N, C]    (row-major)
  S^T = DR(k^T, q^T)                      [N, NQ]   one matmul per m-tile
  es  = exp(S^T/16 - 2) -> fp8 (ACT)
  o^T = DR(v, es) chain                   [C, NQ]
  Z   = DR(ones, es) chain                [16, NQ]  (row 0 used)
  out^T = (Wo^T*2^-0.5).T @ bf16(o^T)     [C, NQ]
  final = xr + out^T * (1/Z),  xr = (x + bo) * 2^-0.5  (host-side)
"""

import numpy as np
import ml_dtypes

import concourse.bass as bass
import concourse.tile as tile
from concourse import bacc, mybir
from concourse.bass_utils import run_bass_kernel_spmd

dt = mybir.dt
F32, F32R, BF16, FP8 = dt.float32, dt.float32r, dt.bfloat16, dt.float8e4
U32 = dt.uint32
AF = mybir.ActivationFunctionType
ALU = mybir.AluOpType
DR = mybir.MatmulPerfMode.DoubleRow

P = 128          # partitions
C = 256          # channels
N = 4096         # tokens per batch element (64*64)
NQ = 2048        # query tokens per core
NSTRIP = 512     # query-token strip width
NS = NQ // NSTRIP  # 4 strips
MT = N // P      # 32 key m-tiles
GS = 8           # channels per group (256 / 32 groups)
EPS = 1e-6
ISCALE = 1.0 / 16.0       # attention scale c**-0.5
EOFF = 2.0                # exp offset: es = exp(s/16 - EOFF), cancels in softmax
RS2 = float(2.0 ** -0.5)  # output residual scale
RSQRT_MAGIC = float(np.frombuffer(np.uint32(0x5F3759DF).tobytes(),
                                  dtype=np.float32)[0])

_prog_cache = {}


def _build_nc():
    nc = bacc.Bacc("TRN2", target_bir_lowering=False, debug=False, num_devices=8)

    def inp(name, shape, d=F32):
        return nc.dram_tensor(name, shape, d, kind="ExternalInput").ap()

    xtb_d = inp("xtb", [2, P, N], BF16)    # [c_half, c_in, n] bf16
    xr_d = inp("xr", [2, P, N])            # (x + bo) * 2^-0.5
    wq_d = inp("wqT", [2, P, C], BF16)     # [ci_half, ci_in, c_out] = Wq.T
    wk_d = inp("wkT", [2, P, C], BF16)
    wv_d = inp("wvT", [2, P, C], BF16)
    wo_d = inp("woT", [2, P, C], BF16)     # Wo.T * 2^-0.5
    bq_d = inp("bqp", [P, 2])              # [c_out_in, c_out_half]
    bk_d = inp("bkp", [P, 2])
    bv_d = inp("bv4", [1, 4 * C])          # bv tiled 4x for [P,4,C] broadcast
    gnw_d = inp("gnw", [P, 2])
    gnb_d = inp("gnb", [P, 2])
    amat_d = inp("amat", [P, P])           # block-diag 8x8 of 1/8
    ones1_d = inp("ones1", [1, P])
    out_d = nc.dram_tensor("out", [2, P, NQ], F32, kind="ExternalOutput").ap()

    with tile.TileContext(nc) as tc:
        with (
            tc.tile_pool(name="singles", bufs=1) as singles,
            tc.tile_pool(name="xpool", bufs=1) as xpool,
            tc.tile_pool(name="hsp", bufs=1) as hsp,
            tc.tile_pool(name="qk", bufs=1) as qk,
            tc.tile_pool(name="vpool", bufs=1) as vpool,
            tc.tile_pool(name="espool", bufs=2) as espool,
            tc.tile_pool(name="small", bufs=2) as small,
            tc.tile_pool(name="zf", bufs=2) as zf,
            tc.tile_pool(name="ps", bufs=2, space="PSUM") as ps,    # 2x2 banks
            tc.tile_pool(name="po", bufs=2, space="PSUM") as po,    # 2x1 bank
            tc.tile_pool(name="pz", bufs=1, space="PSUM") as pz,    # 1 bank
            tc.tile_pool(name="pr", bufs=1, space="PSUM") as pr,    # 1 bank
        ):
            _dmae = [nc.sync, nc.scalar]
            # ---- small consts first: they gate the GN critical path ----
            amat = singles.tile([P, P], F32R)
            nc.sync.dma_start(amat[:], amat_d.bitcast(F32R))
            gnw = singles.tile([P, 2], F32)
            nc.scalar.dma_start(gnw[:], gnw_d)
            gnb = singles.tile([P, 2], F32)
            nc.scalar.dma_start(gnb[:], gnb_d)
            bq = singles.tile([P, 2], F32)
            nc.scalar.dma_start(bq[:], bq_d)
            bk = singles.tile([P, 2], F32)
            nc.scalar.dma_start(bk[:], bk_d)
            ones1 = singles.tile([1, P], F32R)
            nc.scalar.dma_start(ones1[:], ones1_d.bitcast(F32R))
            # bv broadcast to all partitions (stride-0 partition DMA)
            bvrep = singles.tile([P, 4, C], F32)
            bv_b = bass.AP(tensor=bv_d.tensor, offset=bv_d.offset,
                           ap=[[0, P], [1, 4 * C]])
            nc.sync.dma_start(out=bvrep[:].rearrange("p a b -> p (a b)"),
                              in_=bv_b)

            # ---- x load (8 chunks over both hwdge queues) ----
            xtb = xpool.tile([P, 2, N], BF16, tag="xtb")
            for t in range(2):
                for h in range(4):
                    _dmae[h % 2].dma_start(
                        xtb[:, t, h * 1024:(h + 1) * 1024],
                        xtb_d[t, :, h * 1024:(h + 1) * 1024])
            # residual input; needed late (strip tails) so queued after xtb
            xr = xpool.tile([P, 2, N], F32, tag="xr")
            for t in range(2):
                for h in range(2):
                    _dmae[h].dma_start(
                        xr[:, t, h * 2048:(h + 1) * 2048],
                        xr_d[t, :, h * 2048:(h + 1) * 2048])

            # ---- weights via gpsimd software queues (needed ~20us in) ----
            wk = singles.tile([P, 2, C], BF16)
            wq = singles.tile([P, 2, C], BF16)
            wv = singles.tile([P, 2, C], BF16)
            wo = singles.tile([P, 2, C], BF16)
            for w_sb, w_dr in ((wk, wk_d), (wq, wq_d), (wv, wv_d), (wo, wo_d)):
                for ko in range(2):
                    nc.gpsimd.dma_start(w_sb[:, ko, :], w_dr[ko])
            ones8z = singles.tile([P, 2, 16], FP8)
            nc.vector.memset(ones8z[:], 1.0)
            noff = singles.tile([P, 1], F32)
            nc.vector.memset(noff[:], -EOFF)
            epsap = singles.tile([P, 1], F32)
            nc.vector.memset(epsap[:], EPS)
            magic = singles.tile([P, 2], F32)
            nc.vector.memset(magic[:], RSQRT_MAGIC)

            # ---- GroupNorm stats (per channel, then 8-chan group aggregate) ----
            mv2 = small.tile([P, 4], F32, tag="gnmv")  # [mu_t0 mu_t1 ex2_t0 ex2_t1]
            for t in range(2):
                st = small.tile([P, 8, 6], F32, tag="gnst", name=f"gnst{t}")
                xre = xtb[:, t, :].rearrange("p (s f) -> p s f", f=512)
                for sg in range(8):
                    nc.vector.bn_stats(st[:, sg, :], xre[:, sg, :])
                mvt = small.tile([P, 2], F32, tag="gnmvt", name=f"gnmvt{t}")
                nc.vector.bn_aggr(mvt[:], st[:])  # [mean, var]
                musq = small.tile([P, 1], F32, tag="gnmusq", name=f"gnmusq{t}")
                nc.vector.tensor_mul(musq[:], mvt[:, 0:1], mvt[:, 0:1])
                nc.vector.tensor_copy(mv2[:, t:t + 1], mvt[:, 0:1])
                nc.vector.tensor_add(mv2[:, 2 + t:3 + t], mvt[:, 1:2], musq[:])
            stats2 = small.tile([P, 4], F32R, tag="gnst2")
            nc.vector.tensor_copy(stats2[:], mv2[:])
            gp = pz.tile([P, 512], F32, tag="pz", name="gnagg")
            nc.tensor.matmul(gp[:, 0:4], amat[:], stats2[:], start=True, stop=True)
            gs = small.tile([P, 4], F32, tag="gnagg2")
            nc.vector.tensor_copy(gs[:], gp[:, 0:4])
            gmusq = small.tile([P, 2], F32, tag="gnmusq2")
            nc.vector.tensor_mul(gmusq[:], gs[:, 0:2], gs[:, 0:2])
            gvar = small.tile([P, 2], F32, tag="gnvar")
            nc.vector.tensor_tensor(gvar[:], gs[:, 2:4], gmusq[:], ALU.subtract)
            # rstd = rsqrt(var + eps): Quake bit-trick + 2 Newton steps (DVE
            # only — keeps the ACT table on the Exp/Identity set throughout)
            vpe = small.tile([P, 2], F32, tag="gnvpe")
            nc.vector.tensor_scalar(vpe[:], gvar[:], epsap[:], None, ALU.add)
            y0 = small.tile([P, 2], F32, tag="gny0")
            nc.vector.tensor_scalar(y0[:].bitcast(U32), vpe[:].bitcast(U32),
                                    1, None, ALU.logical_shift_right)
            nc.vector.tensor_tensor(y0[:].bitcast(U32), magic[:].bitcast(U32),
                                    y0[:].bitcast(U32), ALU.subtract)
            rstd = small.tile([P, 2], F32, tag="gnrstd")
            tnw = small.tile([P, 2], F32, tag="gnnewt")
            for it in range(2):
                src = y0 if it == 0 else rstd
                dst = rstd if it == 0 else rstd
                nc.vector.tensor_mul(tnw[:], src[:], src[:])
                nc.vector.tensor_mul(tnw[:], tnw[:], vpe[:])
                with nc.allow_low_precision(reason="rsqrt newton step"):
                    nc.vector.tensor_scalar(tnw[:], tnw[:], -0.5, 1.5,
                                            ALU.mult, ALU.add)
                nc.vector.tensor_mul(dst[:], src[:], tnw[:])
            alpha = small.tile([P, 2], F32, tag="gnalpha")
            nc.vector.tensor_mul(alpha[:], rstd[:], gnw[:])
            atmp = small.tile([P, 2], F32, tag="gnatmp")
            nc.vector.tensor_mul(atmp[:], gs[:, 0:2], alpha[:])
            beta = small.tile([P, 2], F32, tag="gnbeta")
            nc.vector.tensor_tensor(beta[:], gnb[:], atmp[:], ALU.subtract)
            hs = hsp.tile([P, 2, N], BF16, tag="hs")
            for h in range(2):
                for t in range(2):
                    nc.vector.tensor_scalar(
                        hs[:, t, h * 2048:(h + 1) * 2048],
                        xtb[:, t, h * 2048:(h + 1) * 2048],
                        alpha[:, t:t + 1], beta[:, t:t + 1],
                        ALU.mult, ALU.add)

            kT = qk.tile([P, 2, N], FP8, tag="kT")
            qT = qk.tile([P, 2, NQ], FP8, tag="qT")

            def emit_proj(wt, bt, dst, blk, on_act, nm):
                # one 512-token block of a q/k projection for both ch halves
                for ch in range(2):
                    kp = po.tile([P, 512], F32, tag="po",
                                 name=f"pj{nm}_{ch}_{blk}")
                    for ko in range(2):
                        nc.tensor.matmul(
                            kp[:], wt[:, ko, ch * P:(ch + 1) * P],
                            hs[:, ko, blk * 512:(blk + 1) * 512],
                            start=(ko == 0), stop=(ko == 1))
                    sl = dst[:, ch, blk * 512:(blk + 1) * 512]
                    if on_act:
                        nc.scalar.activation(sl, kp[:], AF.Identity,
                                             bias=bt[:, ch:ch + 1], scale=1.0)
                    else:
                        nc.vector.tensor_scalar(sl, kp[:], bt[:, ch:ch + 1],
                                                None, ALU.add)

            v = vpool.tile([P, MT, C], FP8)

            def emit_vproj():
                for g in range(MT // 2):
                    vp = po.tile([P, 2, NSTRIP // 2], F32, tag="po",
                                 name=f"vp{g}")
                    for i in range(2):
                        m = 2 * g + i
                        for ko in range(2):
                            nc.tensor.matmul(vp[:, i, :],
                                             hs[:, ko, m * P:(m + 1) * P],
                                             wv[:, ko, :],
                                             start=(ko == 0), stop=(ko == 1))
                    nc.vector.tensor_tensor(v[:, 2 * g:2 * g + 2, :], vp[:],
                                            bvrep[:, 0:2, :], ALU.add)

            # ---- attention strips (software-pipelined emission) ----
            es_t = [None] * NS
            zp_t = [None] * NS
            opa_t = [None] * NS
            opb_t = [None] * NS
            rz_t = [None] * NS
            osb_t = [None] * NS

            def emit_scores_exp(s):
                ns = slice(s * NSTRIP, (s + 1) * NSTRIP)
                es = espool.tile([P, MT, NSTRIP], FP8, tag="es", name=f"es{s}")
                es_t[s] = es
                for j in range(MT // 2):
                    sp = ps.tile([P, 2, NSTRIP], F32, tag="ps", name=f"sp{s}_{j}")
                    for i in range(2):
                        m = 2 * j + i
                        nc.tensor.matmul(sp[:, i, :],
                                         kT[:, :, m * P:(m + 1) * P],
                                         qT[:, :, ns],
                                         start=True, stop=True, perf_mode=DR)
                    nc.scalar.activation(es[:, 2 * j:2 * j + 2, :], sp[:],
                                         AF.Exp, bias=noff[:], scale=ISCALE)

            def emit_zav(s):
                # three accumulation chains, never interleaved (PE constraint)
                es = es_t[s]
                opa = po.tile([P, NSTRIP], F32, tag="po", name=f"opa{s}")
                opb = po.tile([P, NSTRIP], F32, tag="po", name=f"opb{s}")
                zp = pz.tile([P, 512], F32, tag="pz", name=f"zp{s}")
                opa_t[s], opb_t[s], zp_t[s] = opa, opb, zp
                for ch, op in ((0, opa), (1, opb)):
                    for j2 in range(MT // 2):
                        nc.tensor.matmul(op[:],
                                         v[:, 2 * j2:2 * j2 + 2,
                                           ch * P:(ch + 1) * P],
                                         es[:, 2 * j2:2 * j2 + 2, :],
                                         start=(j2 == 0),
                                         stop=(j2 == MT // 2 - 1),
                                         perf_mode=DR)
                for j2 in range(MT // 2):
                    nc.tensor.matmul(zp[0:16, 0:NSTRIP], ones8z[:],
                                     es[:, 2 * j2:2 * j2 + 2, :],
                                     start=(j2 == 0),
                                     stop=(j2 == MT // 2 - 1),
                                     perf_mode=DR)

            def emit_tail_a(s):
                # psum reads that free zp/op for the next strip
                rz = small.tile([1, NSTRIP], F32R, tag="rz", name=f"rz{s}")
                rzf = small.tile([1, NSTRIP], F32, tag="rzf", name=f"rzf{s}")
                rz_t[s] = rz
                osb = small.tile([P, 2, NSTRIP], BF16, tag="osb", name=f"osb{s}")
                osb_t[s] = osb
                nc.vector.tensor_copy(osb[:, 0, :], opa_t[s][:])
                nc.vector.tensor_copy(osb[:, 1, :], opb_t[s][:])
                with nc.allow_low_precision(reason="~18-bit 1/Z is plenty"):
                    nc.vector.reciprocal_approx_fast(rzf[:], zp_t[s][0:1, 0:NSTRIP])
                    nc.vector.tensor_copy(rz[:], rzf[:])

            def emit_tail_b(s):
                ns = slice(s * NSTRIP, (s + 1) * NSTRIP)
                op2 = [po.tile([P, NSTRIP], F32, tag="po", name=f"op2_{s}{ch}")
                       for ch in range(2)]
                for ch in range(2):
                    for ko in range(2):
                        nc.tensor.matmul(op2[ch][:],
                                         wo[:, ko, ch * P:(ch + 1) * P],
                                         osb_t[s][:, ko, :],
                                         start=(ko == 0), stop=(ko == 1))
                rp = pr.tile([P, 512], F32, tag="pr", name=f"rp{s}")
                nc.tensor.matmul(rp[:, 0:NSTRIP], ones1[:],
                                 rz_t[s][:], start=True, stop=True)
                rzs = small.tile([P, NSTRIP], F32, tag="rzs", name=f"rzs{s}")
                nc.vector.tensor_copy(rzs[:], rp[:, 0:NSTRIP])
                tt = zf.tile([P, 2, NSTRIP], F32, tag="tt", name=f"tt{s}")
                for ch in range(2):
                    nc.vector.tensor_tensor(tt[:, ch, :], op2[ch][:],
                                            rzs[:], ALU.mult)
                fin = zf.tile([P, 2, NSTRIP], F32, tag="fin", name=f"fin{s}")
                nc.vector.tensor_tensor(fin[:], xr[:, :, ns], tt[:], ALU.add)
                for t in range(2):
                    nc.sync.dma_start(out_d[t, :, ns], fin[:, t, :])

            # k fully (strip 0 needs all of it), q strip-0 block, then
            # strips interleaved with the rest of the projections so the
            # ACT engine becomes a pure exp stream as early as possible.
            for blk in range(N // 512):
                emit_proj(wk, bk, kT, blk, on_act=True, nm="k")
            emit_proj(wq, bq, qT, 0, on_act=True, nm="q")
            emit_scores_exp(0)
            for blk in range(1, NQ // 512):
                emit_proj(wq, bq, qT, blk, on_act=False, nm="q")
            emit_scores_exp(1)
            emit_vproj()
            for s in range(NS):
                emit_zav(s)
                emit_tail_a(s)
                if s + 2 < NS:
                    emit_scores_exp(s + 2)
                emit_tail_b(s)

    nc.finalize()
    return nc


def _get_nc():
    if "nc" not in _prog_cache:
        _prog_cache["nc"] = _build_nc()
    return _prog_cache["nc"]


def _make_in_maps(x, gn_weight, gn_bias, Wq, bq, Wk, bk, Wv, bv, Wo, bo):
    x = np.asarray(x, dtype=np.float32)
    f32 = lambda a: np.ascontiguousarray(np.asarray(a, dtype=np.float32))
    b16 = lambda a: np.ascontiguousarray(
        np.asarray(a, dtype=np.float32).astype(ml_dtypes.bfloat16))

    def packT(b_vec):  # [256] -> [128, 2] (c_out_in, c_out_half)
        return np.ascontiguousarray(f32(b_vec).reshape(2, P).T)

    amat = np.zeros((P, P), np.float32)
    for g in range(P // GS):
        amat[g * GS:(g + 1) * GS, g * GS:(g + 1) * GS] = 1.0 / GS

    common = {
        "wqT": b16(np.asarray(Wq).T).reshape(2, P, C),
        "wkT": b16(np.asarray(Wk).T).reshape(2, P, C),
        "wvT": b16(np.asarray(Wv).T).reshape(2, P, C),
        "woT": b16(np.asarray(Wo, dtype=np.float32).T * RS2).reshape(2, P, C),
        "bqp": packT(bq),
        "bkp": packT(bk),
        "bv4": np.ascontiguousarray(np.tile(f32(bv).reshape(1, C), (1, 4))),
        "gnw": packT(gn_weight),
        "gnb": packT(gn_bias),
        "amat": amat,
        "ones1": np.ones((1, P), np.float32),
    }

    bo_col = f32(bo).reshape(C, 1)
    in_maps = []
    for core in range(8):
        b, half = core // 2, core % 2
        xt = x[b].reshape(C, N)
        if half:
            xt = np.roll(xt, -NQ, axis=1)
        xrm = ((xt + bo_col) * RS2).astype(np.float32)
        in_maps.append({
            "xtb": np.ascontiguousarray(
                xt.astype(ml_dtypes.bfloat16)).reshape(2, P, N),
            "xr": np.ascontiguousarray(xrm).reshape(2, P, N),
            **common,
        })
    return in_maps


def _assemble(results, B):
    out = np.empty((B, C, N), np.float32)
    for core in range(2 * B):
        b, half = core // 2, core % 2
        out[b, :, half * NQ:(half + 1) * NQ] = results[core]["out"].reshape(C, NQ)
    return out.reshape(B, C, 64, 64)


def kernel(x, gn_weight, gn_bias, Wq, bq, Wk, bk, Wv, bv, Wo, bo):
    x = np.asarray(x, dtype=np.float32)
    in_maps = _make_in_maps(x, gn_weight, gn_bias, Wq, bq, Wk, bk, Wv, bv, Wo, bo)
    nc = _get_nc()
    res = run_bass_kernel_spmd(nc, in_maps, list(range(8)))
    return _assemble(res.results, x.shape[0])


# revision 15
# speedup vs baseline: 1.0564x; 1.0181x over previous
"""AttnBlock (GroupNorm -> single-head self-attention -> residual) on 8 TRN2 cores.

Sharding: B=4 batch elements x 2 query-token halves = 8 cores (SPMD, no
collectives).  Each core receives the full (rolled) channel-major batch
element x^T [C=256, HW=4096] in bf16, computes GroupNorm + k/v for all
4096 tokens, and q/scores/attention/out-proj for its 2048-token half.
Odd cores get x rolled by -2048 tokens; attention is permutation-
invariant over keys, so their first 2048 tokens are tokens 2048:4096.

The two big attention matmuls (scores and attn@v) and the softmax-
denominator chain run in fp8-e4m3 with MatmulPerfMode.DoubleRow
(K=256 packed 2-rows-per-PE-cell, 0.5 cycles/row).  Softmax numerators
use exp(s/16 - 2) so es <= ~57 < 240 (TRN fp8e4 max); the constant
offset cancels in the softmax ratio.  Projections are bf16.  PSUM
accumulation chains never interleave (the PE has one open accumulation
context; interleaving corrupts sums).  GroupNorm rstd uses a Quake-
style rsqrt on DVE (bit-trick + 2 Newton steps) so the ACT engine only
ever loads one table set (Exp/Identity).  Layout is channel-major
(tokens on the free axis), all matmuls transpose-free:

  hs^T = GN(x^T)  bf16                    [C, N]
  q^T = Wq^T.T @ hs^T -> fp8              [C, NQ]   (dim1 = ko ktile)
  k^T likewise -> fp8                     [C, N]
  v   = hs^T.T @ Wv^T + bv -> fp8         [N, C]    (row-major)
  S^T = DR(k^T, q^T)                      [N, NQ]   one matmul per m-tile
  es  = exp(S^T/16 - 2) -> fp8 (ACT)
  o^T = DR(v, es) chain                   [C, NQ]
  Z   = DR(ones, es) chain                [16, NQ]  (row 0 used)
  out^T = (Wo^T*2^-0.5).T @ bf16(o^T)     [C, NQ]
  final = xr + out^T * (1/Z),  xr = (x + bo) * 2^-0.5  (host-side)
"""

import numpy as np
import ml_dtypes

import concourse.bass as bass
import concourse.tile as tile
from concourse import bacc, mybir
from concourse.bass_utils import run_bass_kernel_spmd

dt = mybir.dt
F32, F32R, BF16, FP8 = dt.float32, dt.float32r, dt.bfloat16, dt.float8e4
U32 = dt.uint32
AF = mybir.ActivationFunctionType
ALU = mybir.AluOpType
DR = mybir.MatmulPerfMode.DoubleRow

P = 128          # partitions
C = 256          # channels
N = 4096         # tokens per batch element (64*64)
NQ = 2048        # query tokens per core
NSTRIP = 512     # query-token strip width
NS = NQ // NSTRIP  # 4 strips
MT = N // P      # 32 key m-tiles
GS = 8           # channels per group (256 / 32 groups)
EPS = 1e-6
ISCALE = 1.0 / 16.0       # attention scale c**-0.5
EOFF = 2.0                # exp offset: es = exp(s/16 - EOFF), cancels in softmax
RS2 = float(2.0 ** -0.5)  # output residual scale
RSQRT_MAGIC = float(np.frombuffer(np.uint32(0x5F3759DF).tobytes(),
                                  dtype=np.float32)[0])

_prog_cache = {}


def _build_nc():
    nc = bacc.Bacc("TRN2", target_bir_lowering=False, debug=False, num_devices=8)

    def inp(name, shape, d=F32):
        return nc.dram_tensor(name, shape, d, kind="ExternalInput").ap()

    xtb_d = inp("xtb", [2, P, N], BF16)    # [c_half, c_in, n] bf16
    xr_d = inp("xr", [2, P, N])            # (x + bo) * 2^-0.5
    wq_d = inp("wqT", [2, P, C], BF16)     # [ci_half, ci_in, c_out] = Wq.T
    wk_d = inp("wkT", [2, P, C], BF16)
    wv_d = inp("wvT", [2, P, C], BF16)
    wo_d = inp("woT", [2, P, C], BF16)     # Wo.T * 2^-0.5
    bq_d = inp("bqp", [P, 2])              # [c_out_in, c_out_half]
    bk_d = inp("bkp", [P, 2])
    bv_d = inp("bv4", [1, 4 * C])          # bv tiled 4x for [P,4,C] broadcast
    gnw_d = inp("gnw", [P, 2])
    gnb_d = inp("gnb", [P, 2])
    amat_d = inp("amat", [P, P])           # block-diag 8x8 of 1/8
    ones1_d = inp("ones1", [1, P])
    out_d = nc.dram_tensor("out", [2, P, NQ], F32, kind="ExternalOutput").ap()

    with tile.TileContext(nc) as tc:
        with (
            tc.tile_pool(name="singles", bufs=1) as singles,
            tc.tile_pool(name="xpool", bufs=1) as xpool,
            tc.tile_pool(name="hsp", bufs=1) as hsp,
            tc.tile_pool(name="qk", bufs=1) as qk,
            tc.tile_pool(name="vpool", bufs=1) as vpool,
            tc.tile_pool(name="espool", bufs=2) as espool,
            tc.tile_pool(name="small", bufs=2) as small,
            tc.tile_pool(name="zf", bufs=2) as zf,
            tc.tile_pool(name="ps", bufs=2, space="PSUM") as ps,    # 2x2 banks
            tc.tile_pool(name="po", bufs=2, space="PSUM") as po,    # 2x1 bank
            tc.tile_pool(name="pz", bufs=1, space="PSUM") as pz,    # 1 bank
            tc.tile_pool(name="pr", bufs=1, space="PSUM") as pr,    # 1 bank
        ):
            _dmae = [nc.sync, nc.scalar]
            # ---- small consts first: they gate the GN critical path ----
            amat = singles.tile([P, P], F32R)
            nc.sync.dma_start(amat[:], amat_d.bitcast(F32R))
            gnw = singles.tile([P, 2], F32)
            nc.scalar.dma_start(gnw[:], gnw_d)
            gnb = singles.tile([P, 2], F32)
            nc.scalar.dma_start(gnb[:], gnb_d)
            bq = singles.tile([P, 2], F32)
            nc.scalar.dma_start(bq[:], bq_d)
            bk = singles.tile([P, 2], F32)
            nc.scalar.dma_start(bk[:], bk_d)
            ones1 = singles.tile([1, P], F32R)
            nc.scalar.dma_start(ones1[:], ones1_d.bitcast(F32R))
            # bv broadcast to all partitions (stride-0 partition DMA)
            bvrep = singles.tile([P, 4, C], F32)
            bv_b = bass.AP(tensor=bv_d.tensor, offset=bv_d.offset,
                           ap=[[0, P], [1, 4 * C]])
            nc.sync.dma_start(out=bvrep[:].rearrange("p a b -> p (a b)"),
                              in_=bv_b)

            # ---- x load (8 chunks over both hwdge queues) ----
            xtb = xpool.tile([P, 2, N], BF16, tag="xtb")
            for h in range(4):
                for t in range(2):
                    _dmae[t].dma_start(
                        xtb[:, t, h * 1024:(h + 1) * 1024],
                        xtb_d[t, :, h * 1024:(h + 1) * 1024])
            # residual input; needed late (strip tails) so queued after xtb
            xr = xpool.tile([P, 2, N], F32, tag="xr")
            for t in range(2):
                for h in range(2):
                    _dmae[h].dma_start(
                        xr[:, t, h * 2048:(h + 1) * 2048],
                        xr_d[t, :, h * 2048:(h + 1) * 2048])

            # ---- weights via gpsimd software queues (needed ~20us in) ----
            wk = singles.tile([P, 2, C], BF16)
            wq = singles.tile([P, 2, C], BF16)
            wv = singles.tile([P, 2, C], BF16)
            wo = singles.tile([P, 2, C], BF16)
            for w_sb, w_dr in ((wk, wk_d), (wq, wq_d), (wv, wv_d), (wo, wo_d)):
                for ko in range(2):
                    nc.gpsimd.dma_start(w_sb[:, ko, :], w_dr[ko])
            ones8z = singles.tile([P, 2, 16], FP8)
            nc.vector.memset(ones8z[:], 1.0)
            noff = singles.tile([P, 1], F32)
            nc.vector.memset(noff[:], -EOFF)
            epsap = singles.tile([P, 1], F32)
            nc.vector.memset(epsap[:], EPS)
            magic = singles.tile([P, 2], F32)
            nc.vector.memset(magic[:], RSQRT_MAGIC)

            # ---- GroupNorm stats (per channel, then 8-chan group aggregate) ----
            mv2 = small.tile([P, 4], F32, tag="gnmv")  # [mu_t0 mu_t1 ex2_t0 ex2_t1]
            sts = [small.tile([P, 8, 6], F32, tag="gnst", name=f"gnst{t}")
                   for t in range(2)]
            for h in range(4):
                for t in range(2):
                    xre = xtb[:, t, :].rearrange("p (s f) -> p s f", f=512)
                    for sg in (2 * h, 2 * h + 1):
                        nc.vector.bn_stats(sts[t][:, sg, :], xre[:, sg, :])
            for t in range(2):
                mvt = small.tile([P, 2], F32, tag="gnmvt", name=f"gnmvt{t}")
                nc.vector.bn_aggr(mvt[:], sts[t][:])  # [mean, var]
                musq = small.tile([P, 1], F32, tag="gnmusq", name=f"gnmusq{t}")
                nc.vector.tensor_mul(musq[:], mvt[:, 0:1], mvt[:, 0:1])
                nc.vector.tensor_copy(mv2[:, t:t + 1], mvt[:, 0:1])
                nc.vector.tensor_add(mv2[:, 2 + t:3 + t], mvt[:, 1:2], musq[:])
            stats2 = small.tile([P, 4], F32R, tag="gnst2")
            nc.vector.tensor_copy(stats2[:], mv2[:])
            gp = pz.tile([P, 512], F32, tag="pz", name="gnagg")
            nc.tensor.matmul(gp[:, 0:4], amat[:], stats2[:], start=True, stop=True)
            gs = small.tile([P, 4], F32, tag="gnagg2")
            nc.vector.tensor_copy(gs[:], gp[:, 0:4])
            gmusq = small.tile([P, 2], F32, tag="gnmusq2")
            nc.vector.tensor_mul(gmusq[:], gs[:, 0:2], gs[:, 0:2])
            gvar = small.tile([P, 2], F32, tag="gnvar")
            nc.vector.tensor_tensor(gvar[:], gs[:, 2:4], gmusq[:], ALU.subtract)
            # rstd = rsqrt(var + eps): Quake bit-trick + 2 Newton steps (DVE
            # only — keeps the ACT table on the Exp/Identity set throughout)
            vpe = small.tile([P, 2], F32, tag="gnvpe")
            nc.vector.tensor_scalar(vpe[:], gvar[:], epsap[:], None, ALU.add)
            y0 = small.tile([P, 2], F32, tag="gny0")
            nc.vector.tensor_scalar(y0[:].bitcast(U32), vpe[:].bitcast(U32),
                                    1, None, ALU.logical_shift_right)
            nc.vector.tensor_tensor(y0[:].bitcast(U32), magic[:].bitcast(U32),
                                    y0[:].bitcast(U32), ALU.subtract)
            rstd = small.tile([P, 2], F32, tag="gnrstd")
            tnw = small.tile([P, 2], F32, tag="gnnewt")
            for it in range(2):
                src = y0 if it == 0 else rstd
                dst = rstd if it == 0 else rstd
                nc.vector.tensor_mul(tnw[:], src[:], src[:])
                nc.vector.tensor_mul(tnw[:], tnw[:], vpe[:])
                with nc.allow_low_precision(reason="rsqrt newton step"):
                    nc.vector.tensor_scalar(tnw[:], tnw[:], -0.5, 1.5,
                                            ALU.mult, ALU.add)
                nc.vector.tensor_mul(dst[:], src[:], tnw[:])
            alpha = small.tile([P, 2], F32, tag="gnalpha")
            nc.vector.tensor_mul(alpha[:], rstd[:], gnw[:])
            atmp = small.tile([P, 2], F32, tag="gnatmp")
            nc.vector.tensor_mul(atmp[:], gs[:, 0:2], alpha[:])
            beta = small.tile([P, 2], F32, tag="gnbeta")
            nc.vector.tensor_tensor(beta[:], gnb[:], atmp[:], ALU.subtract)
            hs = hsp.tile([P, 2, N], BF16, tag="hs")

            def emit_hs(h):
                for t in range(2):
                    nc.vector.tensor_scalar(
                        hs[:, t, h * 2048:(h + 1) * 2048],
                        xtb[:, t, h * 2048:(h + 1) * 2048],
                        alpha[:, t:t + 1], beta[:, t:t + 1],
                        ALU.mult, ALU.add)

            kT = qk.tile([P, 2, N], FP8, tag="kT")
            qT = qk.tile([P, 2, NQ], FP8, tag="qT")

            def emit_proj(wt, bt, dst, blk, on_act, nm):
                # one 512-token block of a q/k projection for both ch halves
                for ch in range(2):
                    kp = po.tile([P, 512], F32, tag="po",
                                 name=f"pj{nm}_{ch}_{blk}")
                    for ko in range(2):
                        nc.tensor.matmul(
                            kp[:], wt[:, ko, ch * P:(ch + 1) * P],
                            hs[:, ko, blk * 512:(blk + 1) * 512],
                            start=(ko == 0), stop=(ko == 1))
                    sl = dst[:, ch, blk * 512:(blk + 1) * 512]
                    if on_act:
                        nc.scalar.activation(sl, kp[:], AF.Identity,
                                             bias=bt[:, ch:ch + 1], scale=1.0)
                    else:
                        nc.vector.tensor_scalar(sl, kp[:], bt[:, ch:ch + 1],
                                                None, ALU.add)

            v = vpool.tile([P, MT, C], FP8)

            def emit_vproj():
                for g in range(MT // 2):
                    vp = po.tile([P, 2, NSTRIP // 2], F32, tag="po",
                                 name=f"vp{g}")
                    for i in range(2):
                        m = 2 * g + i
                        for ko in range(2):
                            nc.tensor.matmul(vp[:, i, :],
                                             hs[:, ko, m * P:(m + 1) * P],
                                             wv[:, ko, :],
                                             start=(ko == 0), stop=(ko == 1))
                    nc.vector.tensor_tensor(v[:, 2 * g:2 * g + 2, :], vp[:],
                                            bvrep[:, 0:2, :], ALU.add)

            # ---- attention strips (software-pipelined emission) ----
            es_t = [None] * NS
            zp_t = [None] * NS
            opa_t = [None] * NS
            opb_t = [None] * NS
            rz_t = [None] * NS
            osb_t = [None] * NS

            def emit_scores_exp(s):
                ns = slice(s * NSTRIP, (s + 1) * NSTRIP)
                es = espool.tile([P, MT, NSTRIP], FP8, tag="es", name=f"es{s}")
                es_t[s] = es
                for j in range(MT // 2):
                    sp = ps.tile([P, 2, NSTRIP], F32, tag="ps", name=f"sp{s}_{j}")
                    for i in range(2):
                        m = 2 * j + i
                        nc.tensor.matmul(sp[:, i, :],
                                         kT[:, :, m * P:(m + 1) * P],
                                         qT[:, :, ns],
                                         start=True, stop=True, perf_mode=DR)
                    nc.scalar.activation(
                        es[:, 2 * j:2 * j + 2, :].rearrange("p a b -> p (a b)"),
                        sp[:].rearrange("p a b -> p (a b)"),
                        AF.Exp, bias=noff[:], scale=ISCALE)

            def emit_zav(s):
                # three accumulation chains, never interleaved (PE constraint)
                es = es_t[s]
                opa = po.tile([P, NSTRIP], F32, tag="po", name=f"opa{s}")
                opb = po.tile([P, NSTRIP], F32, tag="po", name=f"opb{s}")
                zp = pz.tile([P, 512], F32, tag="pz", name=f"zp{s}")
                opa_t[s], opb_t[s], zp_t[s] = opa, opb, zp
                for ch, op in ((0, opa), (1, opb)):
                    for j2 in range(MT // 2):
                        nc.tensor.matmul(op[:],
                                         v[:, 2 * j2:2 * j2 + 2,
                                           ch * P:(ch + 1) * P],
                                         es[:, 2 * j2:2 * j2 + 2, :],
                                         start=(j2 == 0),
                                         stop=(j2 == MT // 2 - 1),
                                         perf_mode=DR)
                for j2 in range(MT // 2):
                    nc.tensor.matmul(zp[0:16, 0:NSTRIP], ones8z[:],
                                     es[:, 2 * j2:2 * j2 + 2, :],
                                     start=(j2 == 0),
                                     stop=(j2 == MT // 2 - 1),
                                     perf_mode=DR)

            def emit_tail_a(s):
                # psum reads that free zp/op for the next strip
                rz = small.tile([1, NSTRIP], F32R, tag="rz", name=f"rz{s}")
                rzf = small.tile([1, NSTRIP], F32, tag="rzf", name=f"rzf{s}")
                rz_t[s] = rz
                osb = small.tile([P, 2, NSTRIP], BF16, tag="osb", name=f"osb{s}")
                osb_t[s] = osb
                nc.vector.tensor_copy(osb[:, 0, :], opa_t[s][:])
                nc.vector.tensor_copy(osb[:, 1, :], opb_t[s][:])
                with nc.allow_low_precision(reason="~18-bit 1/Z is plenty"):
                    nc.vector.reciprocal_approx_fast(rzf[:], zp_t[s][0:1, 0:NSTRIP])
                    nc.vector.tensor_copy(rz[:], rzf[:])

            def emit_tail_b(s):
                ns = slice(s * NSTRIP, (s + 1) * NSTRIP)
                op2 = [po.tile([P, NSTRIP], F32, tag="po", name=f"op2_{s}{ch}")
                       for ch in range(2)]
                for ch in range(2):
                    for ko in range(2):
                        nc.tensor.matmul(op2[ch][:],
                                         wo[:, ko, ch * P:(ch + 1) * P],
                                         osb_t[s][:, ko, :],
                                         start=(ko == 0), stop=(ko == 1))
                rp = pr.tile([P, 512], F32, tag="pr", name=f"rp{s}")
                nc.tensor.matmul(rp[:, 0:NSTRIP], ones1[:],
                                 rz_t[s][:], start=True, stop=True)
                rzs = small.tile([P, NSTRIP], F32, tag="rzs", name=f"rzs{s}")
                nc.vector.tensor_copy(rzs[:], rp[:, 0:NSTRIP])
                tt = zf.tile([P, 2, NSTRIP], F32, tag="tt", name=f"tt{s}")
                for ch in range(2):
                    nc.vector.tensor_tensor(tt[:, ch, :], op2[ch][:],
                                            rzs[:], ALU.mult)
                fin = zf.tile([P, 2, NSTRIP], F32, tag="fin", name=f"fin{s}")
                nc.vector.tensor_tensor(fin[:], xr[:, :, ns], tt[:], ALU.add)
                for t in range(2):
                    nc.sync.dma_start(out_d[t, :, ns], fin[:, t, :])

            # ACT = q0 cast + pure exp stream; all other casts on DVE.
            # k-cast blocks trail the scores consumption with margin.
            emit_hs(0)
            emit_proj(wq, bq, qT, 0, on_act=True, nm="q")
            for blk in range(4):
                emit_proj(wk, bk, kT, blk, on_act=False, nm="k")
            emit_hs(1)
            for blk in range(4, 8):
                emit_proj(wk, bk, kT, blk, on_act=False, nm="k")
            emit_scores_exp(0)
            for blk in range(1, NQ // 512):
                emit_proj(wq, bq, qT, blk, on_act=False, nm="q")
            emit_scores_exp(1)
            emit_vproj()
            for s in range(NS):
                emit_zav(s)
                emit_tail_a(s)
                if s + 2 < NS:
                    emit_scores_exp(s + 2)
                emit_tail_b(s)

    nc.finalize()
    return nc


def _get_nc():
    if "nc" not in _prog_cache:
        _prog_cache["nc"] = _build_nc()
    return _prog_cache["nc"]


def _make_in_maps(x, gn_weight, gn_bias, Wq, bq, Wk, bk, Wv, bv, Wo, bo):
    x = np.asarray(x, dtype=np.float32)
    f32 = lambda a: np.ascontiguousarray(np.asarray(a, dtype=np.float32))
    b16 = lambda a: np.ascontiguousarray(
        np.asarray(a, dtype=np.float32).astype(ml_dtypes.bfloat16))

    def packT(b_vec):  # [256] -> [128, 2] (c_out_in, c_out_half)
        return np.ascontiguousarray(f32(b_vec).reshape(2, P).T)

    amat = np.zeros((P, P), np.float32)
    for g in range(P // GS):
        amat[g * GS:(g + 1) * GS, g * GS:(g + 1) * GS] = 1.0 / GS

    common = {
        "wqT": b16(np.asarray(Wq).T).reshape(2, P, C),
        "wkT": b16(np.asarray(Wk).T).reshape(2, P, C),
        "wvT": b16(np.asarray(Wv).T).reshape(2, P, C),
        "woT": b16(np.asarray(Wo, dtype=np.float32).T * RS2).reshape(2, P, C),
        "bqp": packT(bq),
        "bkp": packT(bk),
        "bv4": np.ascontiguousarray(np.tile(f32(bv).reshape(1, C), (1, 4))),
        "gnw": packT(gn_weight),
        "gnb": packT(gn_bias),
        "amat": amat,
        "ones1": np.ones((1, P), np.float32),
    }

    bo_col = f32(bo).reshape(C, 1)
    in_maps = []
    for core in range(8):
        b, half = core // 2, core % 2
        xt = x[b].reshape(C, N)
        if half:
            xt = np.roll(xt, -NQ, axis=1)
        xrm = ((xt + bo_col) * RS2).astype(np.float32)
        in_maps.append({
            "xtb": np.ascontiguousarray(
                xt.astype(ml_dtypes.bfloat16)).reshape(2, P, N),
            "xr": np.ascontiguousarray(xrm).reshape(2, P, N),
            **common,
        })
    return in_maps


def _assemble(results, B):
    out = np.empty((B, C, N), np.float32)
    for core in range(2 * B):
        b, half = core // 2, core % 2
        out[b, :, half * NQ:(half + 1) * NQ] = results[core]["out"].reshape(C, NQ)
    return out.reshape(B, C, 64, 64)


def kernel(x, gn_weight, gn_bias, Wq, bq, Wk, bk, Wv, bv, Wo, bo):
    x = np.asarray(x, dtype=np.float32)
    in_maps = _make_in_maps(x, gn_weight, gn_bias, Wq, bq, Wk, bk, Wv, bv, Wo, bo)
    nc = _get_nc()
    res = run_bass_kernel_spmd(nc, in_maps, list(range(8)))
    return _assemble(res.results, x.shape[0])
